# revision 1
# baseline (speedup 1.0000x reference)
"""nn_CNNxCNN_Attention Trainium2 Bass kernel (self-contained).

Spatial data-parallel sharding over 8 cores (core i: batch i//4, rows
128*(i%4)..+128); the only cross-core exchange is a tiny AllReduce of
per-head Gram matrices + squared norms (L2 normalization folds into the
Gram). Fused 1x1+3x3dw conv via 9 shifted-AP bf16 matmuls in PSUM;
blocked-attention rearrangement via DMA gathers + x-bar DMA transposes;
softmax with norm folding; attn@v and 1x1 proj on the PE.
"""

from dataclasses import dataclass

import numpy as np
import ml_dtypes

import concourse.bass as bass
import concourse.bacc as bacc
import concourse.mybir as mybir
from concourse.tile import TileContext

F32 = mybir.dt.float32
BF16 = mybir.dt.bfloat16
AF = mybir.ActivationFunctionType
ALU = mybir.AluOpType


@dataclass
class Cfg:
    W: int = 512          # image width
    HLOC: int = 128       # rows per core
    n_cores: int = 8
    debug: str = ""       # name of an intermediate to dump, "" = none
    no_collective: bool = False
    no_xpose: bool = False
    stage: int = 5   # truncate pipeline: 1=conv 2=+gram 3=+softmax 4=+attn 5=all
    parts: int = 15  # gram-phase bitmask: 1=gathers 2=xpose 4=mm 8=ttr

    @property
    def W1(self):  # block-cols
        return self.W // 4

    @property
    def YG(self):  # local block-rows
        return self.HLOC // 4

    @property
    def GRP(self):  # block-rows per gram group (= 16-row conv chunk)
        return 4

    @property
    def NG(self):
        return self.YG // self.GRP

    @property
    def NCH(self):  # conv row-chunks of 16 rows
        return self.HLOC // 16


HEADS = 8
CDIM = 6          # channels per head
D = 96            # attention rows per head = CDIM*16
CIN = 48


def build_nc(cfg: Cfg, temps):
    W, W1, HLOC, YG, GRP, NG = cfg.W, cfg.W1, cfg.HLOC, cfg.YG, cfg.GRP, cfg.NG
    HH = HLOC // 2

    nc = bacc.Bacc("TRN2", target_bir_lowering=False, debug=False,
                   num_devices=cfg.n_cores)

    x_in = nc.declare_dram_parameter("x", [CIN, HLOC + 2, W + 2], BF16,
                                     isOutput=False)
    w3qk_in = nc.declare_dram_parameter("w3qk", [CIN, 9, 96], BF16,
                                        isOutput=False)
    w3v_in = nc.declare_dram_parameter("w3v", [CIN, 9, 48], BF16,
                                       isOutput=False)
    wp_in = nc.declare_dram_parameter("wp", [CIN, 48], BF16, isOutput=False)
    id_in = nc.declare_dram_parameter("idm", [128, 128], BF16, isOutput=False)
    y_out = nc.declare_dram_parameter("y", [CIN, HLOC, W], F32, isOutput=True)
    dbg_out = None
    if cfg.debug:
        dbg_out = nc.declare_dram_parameter("dbg", [128, 2048], F32,
                                            isOutput=True)

    cc_in = nc.dram_tensor("cc_in", [96, 784], F32)
    cc_out = nc.dram_tensor("cc_out", [96, 784], F32)
    if not cfg.no_collective:
        half = cfg.n_cores // 2
        groups = [list(range(0, half)), list(range(half, cfg.n_cores))]

    with TileContext(nc) as tc:
        with tc.tile_pool(name="persist", bufs=1) as pp, \
             tc.tile_pool(name="psum_persist", bufs=1, space="PSUM") as ppp:
            # ---- constants ----
            w3qk = pp.tile([112, 5, 96], BF16)
            w3v = pp.tile([112, 5, 48], BF16)
            wp = pp.tile([112, 48], BF16)
            idm = pp.tile([128, 128], BF16)
            # even taps (0,2,4,6,8) at partitions 0-47, odd (1,3,5,7) at 64-111
            nc.sync.dma_start(out=w3qk[0:48, :, :],
                              in_=w3qk_in.rearrange("c (t o) n -> c t o n", o=2)[:, :, 0, :].rearrange("c t n -> c t n") if False else w3qk_in[:, 0:9:2, :])
            nc.sync.dma_start(out=w3qk[64:112, 0:4, :], in_=w3qk_in[:, 1:9:2, :])
            nc.sync.dma_start(out=w3v[0:48, :, :], in_=w3v_in[:, 0:9:2, :])
            nc.sync.dma_start(out=w3v[64:112, 0:4, :], in_=w3v_in[:, 1:9:2, :])
            nc.sync.dma_start(out=wp[0:48, :], in_=wp_in[:])
            nc.sync.dma_start(out=wp[64:112, :], in_=wp_in[:])
            nc.sync.dma_start(out=idm[:], in_=id_in[:])

            # ---- persistent working state ----
            v_sorted = pp.tile([112, 4, 4, HH // 4, W1], BF16)  # a:0-47, b:64-111
            sq_part = pp.tile([96, 16, NG], F32)         # (q8,k8) x group
            gsq = pp.tile([96, 784], F32)                # G(768) + sq(16)

            # ================= conv + gram streaming =================
            with tc.tile_pool(name="conv", bufs=2) as cp, \
                 tc.tile_pool(name="psum_conv", bufs=1, space="PSUM") as pcp:
                xts = {}
                qkbs = {}
                gram_emit = []
                xts[0] = cp.tile([112, 18, W + 2], BF16, tag="xt", name="xt0")
                nc.sync.dma_start(out=xts[0][0:48], in_=x_in[:, 0:18, :])
                nc.sync.dma_start(out=xts[0][64:112], in_=x_in[:, 0:18, :])
                for g in range(NG):
                    if g + 1 < NG:
                        xts[g + 1] = cp.tile([112, 18, W + 2], BF16,
                                             tag="xt", name=f"xt{g+1}")
                        nc.sync.dma_start(
                            out=xts[g + 1][0:48],
                            in_=x_in[:, 16 * (g + 1):16 * (g + 1) + 18, :])
                        nc.sync.dma_start(
                            out=xts[g + 1][64:112],
                            in_=x_in[:, 16 * (g + 1):16 * (g + 1) + 18, :])
                    xt = xts[g]
                    qkb = cp.tile([96, 4, 4, GRP, W1], BF16, tag="qkb")
                    qkbs[g] = qkb
                    for yy in range(16):
                        y = 16 * g + yy
                        ps_qk = pcp.tile([96, W], F32, tag="ps_qk",
                                          bufs=2)
                        ps_qk2 = pcp.tile([96, W], F32, tag="ps_qk2",
                                           bufs=2)
                        ps_v = pcp.tile([112, W], F32, tag="ps_v")
                        ps_v2 = pcp.tile([112, W], F32, tag="ps_v2")
                        vbase = 0 if y < HH else 64
                        vs = ps_v[vbase:vbase + 48, :]
                        vs2 = ps_v2[vbase:vbase + 48, :]
                        for p in range(5):
                            tA, tB = 2 * p, 2 * p + 1
                            dyA, dxA = tA // 3, tA % 3
                            rhsA = xt[0:48, yy + dyA, dxA:dxA + W]
                            nc.tensor.matmul(ps_qk[:], w3qk[0:48, p, :], rhsA,
                                             start=(p == 0), stop=(p == 4))
                            if tB < 9:
                                dyB, dxB = tB // 3, tB % 3
                                rhsB = xt[64:112, yy + dyB, dxB:dxB + W]
                                nc.tensor.matmul(
                                    ps_qk2[:], w3qk[64:112, p, :], rhsB,
                                    start=(p == 0), stop=(p == 3),
                                    tile_position=(64, 0))
                        for p in range(5):
                            tA, tB = 2 * p, 2 * p + 1
                            dyA, dxA = tA // 3, tA % 3
                            rhsA = xt[0:48, yy + dyA, dxA:dxA + W]
                            nc.tensor.matmul(vs, w3v[0:48, p, :], rhsA,
                                             start=(p == 0), stop=(p == 4),
                                             tile_position=(0, vbase))
                            if tB < 9:
                                dyB, dxB = tB // 3, tB % 3
                                rhsB = xt[64:112, yy + dyB, dxB:dxB + W]
                                nc.tensor.matmul(
                                    vs2, w3v[64:112, p, :], rhsB,
                                    start=(p == 0), stop=(p == 3),
                                    tile_position=(64, vbase))
                        yg_l, nh = yy // 4, yy % 4
                        # unsort x=(4*w1+nw) -> (nw, w1); sum the two banks
                        src_qk = ps_qk.rearrange("p (w1 nw) -> p nw w1", nw=4)
                        src_qk2 = ps_qk2.rearrange("p (w1 nw) -> p nw w1", nw=4)
                        dst_qk = qkb[:, nh, :, yg_l, :]
                        nc.scalar.copy(dst_qk, src_qk)
                        nc.vector.tensor_tensor(dst_qk, dst_qk, src_qk2,
                                                op=ALU.add)
                        src_v = vs.rearrange("p (w1 nw) -> p nw w1", nw=4)
                        src_v2 = vs2.rearrange("p (w1 nw) -> p nw w1", nw=4)
                        dst_v = v_sorted[vbase:vbase + 48, nh, :,
                                         (y % HH) // 4, :]
                        nc.scalar.copy(dst_v, src_v)
                        nc.vector.tensor_tensor(dst_v, dst_v, src_v2,
                                                op=ALU.add)

                    # ---- gram for group g-1 (pipelined one behind) ----
                    def emit_gram(g, qkb):
                        for h in range(HEADS if cfg.stage >= 2 else 0):
                            qcb = cp.tile([96, GRP, W1], BF16, tag="qcb")
                            kcb = cp.tile([96, GRP, W1], BF16, tag="kcb")
                            if cfg.parts & 1:
                                nc.scalar.dma_start(out=qcb[:],
                                                    in_=qkb[6 * h:6 * h + 6])
                                nc.scalar.dma_start(
                                    out=kcb[:],
                                    in_=qkb[48 + 6 * h:48 + 6 * h + 6])
                            else:
                                nc.vector.memset(qcb[:], 0.125)
                                nc.vector.memset(kcb[:], 0.125)
                            NCK = GRP * W1 // 128
                            qT = cp.tile([128, NCK, 96], BF16, tag="qT")
                            kT = cp.tile([128, NCK, 96], BF16, tag="kT")
                            qcb_f = qcb.rearrange("p a b -> p (a b)")
                            kcb_f = kcb.rearrange("p a b -> p (a b)")
                            for ck in range(NCK):
                                tq = pcp.tile([128, 96], BF16, tag="tps")
                                nc.tensor.transpose(
                                    tq[:], qcb_f[:, 128 * ck:128 * ck + 128],
                                    idm[0:96, 0:96])
                                nc.vector.tensor_copy(qT[:, ck, :], tq[:])
                                tk_ = pcp.tile([128, 96], BF16, tag="tps")
                                nc.tensor.transpose(
                                    tk_[:], kcb_f[:, 128 * ck:128 * ck + 128],
                                    idm[0:96, 0:96])
                                nc.vector.tensor_copy(kT[:, ck, :], tk_[:])
                            gps = pcp.tile([96, 96], F32, tag="gps")
                            for ck in range(NCK):
                                nc.tensor.matmul(
                                    gps[:], kT[:, ck, :], qT[:, ck, :],
                                    start=(ck == 0), stop=(ck == NCK - 1))
                            if g == 0:
                                nc.vector.tensor_copy(
                                    gsq[:, 96 * h:96 * h + 96], gps[:])
                            else:
                                nc.vector.tensor_tensor(
                                    gsq[:, 96 * h:96 * h + 96],
                                    gsq[:, 96 * h:96 * h + 96], gps[:],
                                    op=ALU.add)
                            if cfg.parts & 8:
                                sc0 = cp.tile([96, GRP * W1], BF16, tag="sc0")
                                nc.scalar.activation(
                                    sc0[:], qcb.rearrange("p a b -> p (a b)"),
                                    AF.Square,
                                    accum_out=sq_part[:, h, g:g + 1])
                                sc1 = cp.tile([96, GRP * W1], BF16, tag="sc1")
                                nc.scalar.activation(
                                    sc1[:], kcb.rearrange("p a b -> p (a b)"),
                                    AF.Square,
                                    accum_out=sq_part[:, 8 + h, g:g + 1])
                            else:
                                nc.vector.memset(sq_part[:, h, g:g+1], 1.0)
                                nc.vector.memset(sq_part[:, 8+h, g:g+1], 1.0)
                    if g > 0:
                        emit_gram(g - 1, qkbs.pop(g - 1))
                emit_gram(NG - 1, qkbs.pop(NG - 1))

            # ================= collective =================
            if cfg.stage >= 3:
                _tail(nc, tc, cfg, pp, v_sorted, sq_part, gsq, idm, wp,
                      cc_in, cc_out, groups if not cfg.no_collective else None,
                      y_out, temps, dbg_out)
            else:
                with tc.tile_pool(name="dummy", bufs=1) as dp:
                    zt = dp.tile([CIN, W], F32)
                    nc.vector.memset(zt[:], 0.0)
                    for yy in range(HLOC):
                        nc.sync.dma_start(out=y_out[:, yy, :], in_=zt[:])

    nc.compile()
    nc.generate_event_semaphores()
    return nc


def _tail(nc, tc, cfg, pp, v_sorted, sq_part, gsq, idm, wp, cc_in, cc_out,
          groups, y_out, temps, dbg_out):
    W, W1, HLOC, YG = cfg.W, cfg.W1, cfg.HLOC, cfg.YG
    HH = HLOC // 2
    with tc.tile_pool(name="tailp", bufs=1) as tp:
        if True:
            nc.vector.tensor_reduce(gsq[:, 768:784], sq_part[:],
                                    axis=mybir.AxisListType.X, op=ALU.add)
            nc.sync.dma_start(out=cc_in[:], in_=gsq[:])
            if groups is None:
                nc.sync.dma_start(out=cc_out[:], in_=cc_in[:])
            else:
                nc.gpsimd.collective_compute(
                    "AllReduce", ALU.add, replica_groups=groups,
                    ins=[cc_in[:]], outs=[cc_out[:]])
            gsq_r = tp.tile([96, 784], F32)
            nc.sync.dma_start(out=gsq_r[:], in_=cc_out[:])
            if cfg.debug == "gsq":
                nc.sync.dma_start(out=dbg_out[0:96, 0:784], in_=gsq_r[:])

            # ================= softmax + attn@v + proj =================
            out_sorted = tp.tile([112, 4, 4, HH // 4, W1], BF16)
            with tc.tile_pool(name="attn", bufs=2) as ap, \
                 tc.tile_pool(name="psum_attn", bufs=2, space="PSUM") as pap:
                # norms: rs = 1/max(sqrt(sq), 1e-12)
                nrm = tp.tile([96, 16], F32)
                rs = tp.tile([96, 16], F32)
                nc.scalar.sqrt(nrm[:], gsq_r[:, 768:784])
                nc.vector.tensor_scalar_max(nrm[:], nrm[:], 1e-12)
                nc.vector.reciprocal(rs[:], nrm[:])

                vcbs = {}
                if cfg.stage >= 4:
                    for h in range(HEADS):
                        vcbs[h] = ap.tile([96, YG, W1], BF16, tag="vcb",
                                          bufs=4, name=f"vcb{h}")
                        for bi, vb in ((0, 0), (1, 64)):
                            dst = vcbs[h][:, bi * (YG // 2):
                                          (bi + 1) * (YG // 2), :]
                            nc.sync.dma_start(
                                out=dst,
                                in_=v_sorted[vb + 6 * h:vb + 6 * h + 6])
                for h in range(HEADS):
                    # H = G_T * rs_k[e] (rows e), bf16
                    hbf = ap.tile([96, 96], BF16, tag="hbf")
                    nc.vector.tensor_scalar_mul(
                        hbf[:], gsq_r[:, 96 * h:96 * h + 96],
                        rs[:, 8 + h:8 + h + 1])
                    ht_ps = pap.tile([96, 96], BF16, tag="ht_ps")
                    nc.tensor.transpose(ht_ps[:], hbf[:], idm[0:96, 0:96])
                    # s_d = temp[h] * rs_q[d]
                    sd = ap.tile([96, 1], F32, tag="sd")
                    nc.vector.tensor_scalar_mul(sd[:], rs[:, h:h + 1],
                                                float(temps[h]))
                    aexp = ap.tile([96, 96], BF16, tag="aexp")
                    rowsum = ap.tile([96, 1], F32, tag="rowsum")
                    nc.scalar.activation(aexp[:], ht_ps[:], AF.Exp,
                                         scale=sd[:], accum_out=rowsum[:])
                    at_ps = pap.tile([96, 96], BF16, tag="at_ps")
                    nc.tensor.transpose(at_ps[:], aexp[:], idm[0:96, 0:96])
                    atT = ap.tile([96, 96], BF16, tag="atT")
                    nc.scalar.copy(atT[:], at_ps[:])
                    rinv = ap.tile([96, 1], F32, tag="rinv")
                    nc.vector.reciprocal(rinv[:], rowsum[:])

                    if cfg.stage < 4:
                        continue
                    vcb = vcbs[h]

                    ocb = ap.tile([96, YG, W1], BF16, tag="ocb")
                    ntok = YG * W1
                    ncnk = max(1, ntok // 512)
                    csz = ntok // ncnk  # tokens per chunk (<=512)
                    vcb_f = vcb.rearrange("p a b -> p (a b)")
                    ocb_f = ocb.rearrange("p a b -> p (a b)")
                    for ck in range(ncnk):
                        av_ps = pap.tile([96, csz], F32, tag="av_ps")
                        nc.tensor.matmul(av_ps[:], atT[:],
                                         vcb_f[:, ck * csz:(ck + 1) * csz],
                                         start=True, stop=True)
                        nc.vector.tensor_scalar_mul(
                            ocb_f[:, ck * csz:(ck + 1) * csz], av_ps[:],
                            rinv[:])
                    # scatter to out_sorted halves
                    for bi, vb in ((0, 0), (1, 64)):
                        src = ocb[:, bi * (YG // 2):(bi + 1) * (YG // 2), :]
                        nc.sync.dma_start(
                            out=out_sorted[vb + 6 * h:vb + 6 * h + 6], in_=src)

            # ---- proj ----
            if cfg.stage < 5:
                with tc.tile_pool(name="dummy", bufs=1) as dp:
                    zt = dp.tile([CIN, W], F32)
                    nc.vector.memset(zt[:], 0.0)
                    for yy in range(HLOC):
                        nc.sync.dma_start(out=y_out[:, yy, :], in_=zt[:])
                return
            with tc.tile_pool(name="proj", bufs=2) as prp, \
                 tc.tile_pool(name="psum_proj", bufs=2, space="PSUM") as pprp:
                for yc in range(HLOC // 8):
                    stg = prp.tile([48, 8, W], F32, tag="stg")
                    for yy in range(8):
                        y = 8 * yc + yy
                        vbase = 0 if y < HH else 64
                        rhs = out_sorted[vbase:vbase + 48, y % 4, :,
                                         (y % HH) // 4, :]
                        rhs = rhs.rearrange("p nw w1 -> p w1 nw")  # x-order
                        pj = pprp.tile([48, W], F32, tag="pj")
                        nc.tensor.matmul(pj[:], wp[vbase:vbase + 48, :],
                                         rhs, start=True, stop=True)
                        nc.scalar.copy(stg[:, yy, :], pj[:])
                    nc.sync.dma_start(out=y_out[:, 8 * yc:8 * yc + 8, :],
                                      in_=stg[:])


# ====================== host-side helpers ======================

def host_prepare(x, qkv_w, dw_w, proj_w, cfg: Cfg):
    """Full inputs -> list of per-core input dicts."""
    bf = ml_dtypes.bfloat16
    b, cin, H, W = x.shape
    assert cin == CIN and W == cfg.W
    n = cfg.n_cores
    per_b = n // b          # cores per batch
    assert cfg.HLOC * per_b == H

    w1 = qkv_w[:, :, 0, 0].astype(np.float32)        # [144, 48]
    dw = dw_w[:, 0, :, :].reshape(144, 9).astype(np.float32)
    w3 = w1[:, None, :] * dw[:, :, None]             # [144, 9, 48]
    w3T = np.ascontiguousarray(w3.transpose(2, 1, 0))  # [48, 9, 144]
    w3qk = w3T[:, :, 0:96].astype(bf)
    w3v = w3T[:, :, 96:144].astype(bf)
    wp = np.ascontiguousarray(proj_w[:, :, 0, 0].astype(np.float32).T
                              ).astype(bf)           # [48in, 48out]
    idm = np.eye(128, dtype=np.float32).astype(bf)

    xp = np.zeros((b, cin, H + 2, W + 2), dtype=np.float32)
    xp[:, :, 1:H + 1, 1:W + 1] = x
    xp = xp.astype(bf)

    in_maps = []
    for i in range(n):
        bi, r = i // per_b, i % per_b
        xs = np.ascontiguousarray(
            xp[bi, :, cfg.HLOC * r:cfg.HLOC * r + cfg.HLOC + 2, :])
        in_maps.append({"x": xs, "w3qk": w3qk, "w3v": w3v, "wp": wp,
                        "idm": idm})
    return in_maps


def host_assemble(results, cfg: Cfg, b, H):
    n = cfg.n_cores
    per_b = n // b
    y = np.empty((b, CIN, H, cfg.W), dtype=np.float32)
    for i in range(n):
        bi, r = i // per_b, i % per_b
        y[bi, :, cfg.HLOC * r:cfg.HLOC * (r + 1), :] = results[i]["y"]
    return y


_NC_CACHE = {}


def kernel(x, qkv_w, dw_w, proj_w, temperature):
    import numpy as _np
    x = _np.asarray(x, dtype=_np.float32)
    qkv_w = _np.asarray(qkv_w, dtype=_np.float32)
    dw_w = _np.asarray(dw_w, dtype=_np.float32)
    proj_w = _np.asarray(proj_w, dtype=_np.float32)
    temperature = _np.asarray(temperature, dtype=_np.float32)

    from concourse.bass_utils import run_bass_kernel_spmd

    cfg = Cfg(W=512, HLOC=128, n_cores=8)
    temps = tuple(float(t) for t in temperature[:, 0, 0])
    if temps not in _NC_CACHE:
        _NC_CACHE[temps] = build_nc(cfg, list(temps))
    nc = _NC_CACHE[temps]
    in_maps = host_prepare(x, qkv_w, dw_w, proj_w, cfg)
    res = run_bass_kernel_spmd(nc, in_maps, list(range(cfg.n_cores)))
    return host_assemble(res.results, cfg, x.shape[0], x.shape[2])



# revision 17
# speedup vs baseline: 1.3292x; 1.3292x over previous
"""nn_CNNxCNN_Attention Trainium2 Bass kernel (self-contained).

Row-sharded data parallelism over 8 cores (core i: batch i//4, rows
128*(i%4)..+128); the only cross-core exchange is an AllReduce of
per-head Gram matrices + squared norms (L2 normalization folds into
the Gram).

Fused 1x1+3x3dw conv via K-packed bf16 matmuls: x is staged in SBUF
2.67x (three column-shifted copies on partitions 0-47/48-95/96-127 plus
a small row-shifted remainder tile) so each matmul contracts K=128 over
(channel x tap) -- 4 matmuls per 512-wide output row per output half
(qk 96 / v 48) instead of 18 K=48 matmuls.

Blocked-attention per-head expansion via DMA gathers; q/k chunk
transposes via the DMA crossbar transpose (dma_start_transpose), not
the PE; Gram + squared norms on the PE; softmax with norm and 1/rowsum
folding; attn@v output regrouped by DMA into (head,chan)-partition
layout so the 1x1 proj runs with contiguous rhs; y written HBM-blocked
and un-blocked on the host.
"""

from dataclasses import dataclass

import numpy as np
import ml_dtypes

import concourse.bass as bass
import concourse.bacc as bacc
import concourse.mybir as mybir
from concourse.tile import TileContext

F32 = mybir.dt.float32
BF16 = mybir.dt.bfloat16
AF = mybir.ActivationFunctionType
ALU = mybir.AluOpType

HEADS = 8
CIN = 48
W = 512
W1 = 128          # coarse token columns
HLOC = 128        # rows per core
NSB = 4           # superblocks of 32 rows
SBROWS = 32
NCK = 8           # attn/proj token chunks (4 h1-rows of 128 w1 each)


@dataclass
class Cfg:
    W: int = 512
    HLOC: int = 128
    n_cores: int = 8


def build_nc(temps, n_cores=8, stage=3, parts=15):
    # stage: 1=conv+unsort only, 2=+gram (zeros out), 3=full
    # parts bitmask (gram sub-stages): 1=gathers 2=transposes 4=gram-mm 8=squares
    nc = bacc.Bacc("TRN2", target_bir_lowering=False, debug=False,
                   num_devices=n_cores)

    x_in = nc.declare_dram_parameter("x", [CIN, HLOC + 2, W + 2], BF16,
                                     isOutput=False)
    w3a_in = nc.declare_dram_parameter("w3a", [128, 3, 96], BF16,
                                       isOutput=False)
    w3av_in = nc.declare_dram_parameter("w3av", [128, 3, 48], BF16,
                                        isOutput=False)
    w3b_in = nc.declare_dram_parameter("w3b", [48, 96], BF16, isOutput=False)
    w3bv_in = nc.declare_dram_parameter("w3bv", [48, 48], BF16,
                                        isOutput=False)
    wp_in = nc.declare_dram_parameter("wp", [48, 48], BF16, isOutput=False)
    id_in = nc.declare_dram_parameter("idm", [128, 128], BF16, isOutput=False)
    # blocked output: [ch, nh, nw, h1loc, w1]; host re-interleaves
    y_out = nc.declare_dram_parameter("y", [CIN, 4, 4, 32, 128], F32,
                                      isOutput=True)

    cc_in = nc.dram_tensor("cc_in", [96, 784], F32)
    cc_out = nc.dram_tensor("cc_out", [96, 784], F32)
    half = n_cores // 2
    groups = [list(range(0, half)), list(range(half, n_cores))]

    with TileContext(nc) as tc:
        with tc.tile_pool(name="persist", bufs=1) as pp:
            # ---- constants ----
            w3a = pp.tile([128, 3, 96], BF16)
            w3av = pp.tile([128, 3, 48], BF16)
            w3b = pp.tile([48, 96], BF16)
            w3bv = pp.tile([48, 48], BF16)
            wp = pp.tile([48, 48], BF16)
            idm = pp.tile([128, 128], BF16)
            nc.sync.dma_start(out=w3a[:], in_=w3a_in[:])
            nc.sync.dma_start(out=w3av[:], in_=w3av_in[:])
            nc.sync.dma_start(out=w3b[:], in_=w3b_in[:])
            nc.sync.dma_start(out=w3bv[:], in_=w3bv_in[:])
            nc.sync.dma_start(out=wp[:], in_=wp_in[:])
            nc.sync.dma_start(out=idm[:], in_=id_in[:])

            # ---- persistent working state ----
            vcbs = {}
            for h in range(HEADS):
                vcbs[h] = pp.tile([96, 32, W1], BF16, name=f"vcb{h}")
            sq_part = pp.tile([96, 16, NSB], F32)   # (q8,k8) x superblock
            gsq = pp.tile([96, 784], F32)           # G(768) + sq(16)

            # ================= conv + gram streaming =================
            with tc.tile_pool(name="conv", bufs=2) as cp, \
                 tc.tile_pool(name="psum_conv", bufs=1, space="PSUM") as pcp:
                xrs, xr2s = {}, {}

                def load_group(g):
                    # xr: parts 0-47 = x (dx tap 0), 48-95 = x shifted one
                    # col (dx tap 1), 96-127 = x[0:32] shifted two (dx tap 2)
                    xr = cp.tile([128, 18, 512], BF16, tag="xr")
                    r0 = 16 * g
                    nc.sync.dma_start(out=xr[0:48, :, :],
                                      in_=x_in[:, r0:r0 + 18, 0:512])
                    nc.sync.dma_start(out=xr[48:96, :, :],
                                      in_=x_in[:, r0:r0 + 18, 1:513])
                    nc.sync.dma_start(out=xr[96:128, :, :],
                                      in_=x_in[0:32, r0:r0 + 18, 2:514])
                    # xr2: remainder taps (dy,2) for channels 32-47,
                    # partition block dy holds rows r0+dy+hh*8 .. +8
                    halves = []
                    for hh in range(2):
                        x2 = cp.tile([48, 8, 512], BF16, tag="xr2")
                        for dy in range(3):
                            rb = r0 + dy + 8 * hh
                            nc.scalar.dma_start(
                                out=x2[16 * dy:16 * dy + 16, :, :],
                                in_=x_in[32:48, rb:rb + 8, 2:514])
                        halves.append(x2)
                    xrs[g] = xr
                    xr2s[g] = halves

                load_group(0)
                for sb in range(NSB):
                    # free layout: (nh, nw, gg, grprow, w1) so per-head
                    # gathers read fully contiguous runs
                    qkb = cp.tile([96, 4, 4, 2, 4, W1], BF16, tag="qkb",
                                  bufs=1)
                    vstg = cp.tile([48, 4, 4, 2, 4, W1], BF16, tag="vstg",
                                   bufs=1)
                    for gg in range(2):
                        g = 2 * sb + gg
                        xr, xr2 = xrs.pop(g), xr2s.pop(g)
                        for yy in range(16):
                            if yy == 8 and g + 1 < 2 * NSB:
                                load_group(g + 1)
                            x2 = xr2[yy // 8]
                            yl = yy % 8
                            ps_qk = pcp.tile([96, W], F32, tag="ps_qk",
                                             bufs=2)
                            ps_v = pcp.tile([48, W], F32, tag="ps_v", bufs=2)
                            for dy in range(3):
                                rhs = xr[0:128, yy + dy, 0:512]
                                nc.tensor.matmul(ps_qk[:], w3a[:, dy, :], rhs,
                                                 start=(dy == 0), stop=False)
                            nc.tensor.matmul(ps_qk[:], w3b[:],
                                             x2[0:48, yl, :],
                                             start=False, stop=True)
                            for dy in range(3):
                                rhs = xr[0:128, yy + dy, 0:512]
                                nc.tensor.matmul(ps_v[:], w3av[:, dy, :], rhs,
                                                 start=(dy == 0), stop=False)
                            nc.tensor.matmul(ps_v[:], w3bv[:],
                                             x2[0:48, yl, :],
                                             start=False, stop=True)
                            nh, grprow = yy % 4, yy // 4
                            src_qk = ps_qk.rearrange("p (w1 nw) -> p nw w1",
                                                     nw=4)
                            nc.scalar.copy(qkb[:, nh, :, gg, grprow, :],
                                           src_qk)
                            src_v = ps_v.rearrange("p (w1 nw) -> p nw w1",
                                                   nw=4)
                            nc.vector.tensor_copy(vstg[:, nh, :, gg,
                                                        grprow, :], src_v)

                    # ---- per-superblock: vcb gathers + gram ----
                    for h in range(HEADS):
                        nc.sync.dma_start(
                            out=vcbs[h][:, 8 * sb:8 * sb + 8, :],
                            in_=vstg[6 * h:6 * h + 6])
                    for h in range(HEADS if stage >= 2 else 0):
                        qcb = cp.tile([96, 8, W1], BF16, tag="qcb")
                        kcb = cp.tile([96, 8, W1], BF16, tag="kcb")
                        if parts & 1:
                            nc.scalar.dma_start(out=qcb[:],
                                                in_=qkb[6 * h:6 * h + 6])
                            nc.scalar.dma_start(
                                out=kcb[:],
                                in_=qkb[48 + 6 * h:48 + 6 * h + 6])
                        else:
                            nc.vector.memset(qcb[:], 0.125)
                            nc.vector.memset(kcb[:], 0.125)
                        qT = cp.tile([128, 8, 96], BF16, tag="qT")
                        kT = cp.tile([128, 8, 96], BF16, tag="kT")
                        if parts & 2:
                            nc.sync.dma_start_transpose(
                                qT[:], qcb.rearrange("p a b -> p (a b)"))
                            nc.sync.dma_start_transpose(
                                kT[:], kcb.rearrange("p a b -> p (a b)"))
                        else:
                            nc.vector.memset(qT[:], 0.125)
                            nc.vector.memset(kT[:], 0.125)
                        gps = pcp.tile([96, 96], F32, tag="gps", bufs=2)
                        if parts & 4:
                            for ck in range(8):
                                nc.tensor.matmul(gps[:], kT[:, ck, :],
                                                 qT[:, ck, :],
                                                 start=(ck == 0),
                                                 stop=(ck == 7))
                        else:
                            nc.tensor.matmul(gps[:], kT[:, 0, :], qT[:, 0, :],
                                             start=True, stop=True)
                        if sb == 0:
                            nc.vector.tensor_copy(gsq[:, 96 * h:96 * h + 96],
                                                  gps[:])
                        else:
                            nc.vector.tensor_tensor(
                                gsq[:, 96 * h:96 * h + 96],
                                gsq[:, 96 * h:96 * h + 96], gps[:],
                                op=ALU.add)
                        sc0 = cp.tile([96, 8 * W1], BF16, tag="sc0")
                        if parts & 8:
                            nc.scalar.activation(
                                sc0[:], qcb.rearrange("p a b -> p (a b)"),
                                AF.Square,
                                accum_out=sq_part[:, h, sb:sb + 1])
                            sc1 = cp.tile([96, 8 * W1], BF16, tag="sc0")
                            nc.scalar.activation(
                                sc1[:], kcb.rearrange("p a b -> p (a b)"),
                                AF.Square,
                                accum_out=sq_part[:, 8 + h, sb:sb + 1])
                        else:
                            nc.vector.memset(sq_part[:, h, sb:sb + 1], 1.0)
                            nc.vector.memset(sq_part[:, 8 + h, sb:sb + 1],
                                             1.0)

            # ================= collective =================
            if stage < 3:
                with tc.tile_pool(name="zf", bufs=1) as zp:
                    zt = zp.tile([CIN, 4, 4, W], F32)
                    nc.vector.memset(zt[:], 0.0)
                    for ck in range(NCK):
                        nc.sync.dma_start(
                            out=y_out[:, :, :, 4 * ck:4 * ck + 4, :],
                            in_=zt[:])
            if stage >= 3:
              with tc.tile_pool(name="tailp", bufs=1) as tp:
                nc.vector.tensor_reduce(gsq[:, 768:784], sq_part[:],
                                        axis=mybir.AxisListType.X, op=ALU.add)
                nc.sync.dma_start(out=cc_in[:], in_=gsq[:])
                nc.gpsimd.collective_compute(
                    "AllReduce", ALU.add, replica_groups=groups,
                    ins=[cc_in[:]], outs=[cc_out[:]])
                gsq_r = tp.tile([96, 784], F32)
                nc.sync.dma_start(out=gsq_r[:], in_=cc_out[:])

                # ============ softmax (norm + 1/rowsum folded) ============
                nrm = tp.tile([96, 16], F32)
                rs = tp.tile([96, 16], F32)
                nc.scalar.sqrt(nrm[:], gsq_r[:, 768:784])
                nc.vector.tensor_scalar_max(nrm[:], nrm[:], 1e-12)
                nc.vector.reciprocal(rs[:], nrm[:])

                atTs = {}
                with tc.tile_pool(name="smx", bufs=2) as sp, \
                     tc.tile_pool(name="psum_smx", bufs=2,
                                  space="PSUM") as psp:
                    for h in range(HEADS):
                        # gsq holds G^T[e,d]; scale rows (e) by k-norms
                        hbf = sp.tile([96, 96], BF16, tag="hbf")
                        nc.vector.tensor_scalar_mul(
                            hbf[:], gsq_r[:, 96 * h:96 * h + 96],
                            rs[:, 8 + h:8 + h + 1])
                        ht_ps = psp.tile([96, 96], BF16, tag="ht_ps")
                        nc.tensor.transpose(ht_ps[:], hbf[:], idm[0:96, 0:96])
                        sd = sp.tile([96, 1], F32, tag="sd")
                        nc.vector.tensor_scalar_mul(sd[:], rs[:, h:h + 1],
                                                    float(temps[h]))
                        aexp = sp.tile([96, 96], BF16, tag="aexp")
                        rowsum = sp.tile([96, 1], F32, tag="rowsum")
                        nc.scalar.activation(aexp[:], ht_ps[:], AF.Exp,
                                             scale=sd[:], accum_out=rowsum[:])
                        rinv = sp.tile([96, 1], F32, tag="rinv")
                        nc.vector.reciprocal(rinv[:], rowsum[:])
                        aexp2 = sp.tile([96, 96], BF16, tag="aexp2")
                        nc.vector.tensor_scalar_mul(aexp2[:], aexp[:],
                                                    rinv[:])
                        at_ps = psp.tile([96, 96], BF16, tag="at_ps")
                        nc.tensor.transpose(at_ps[:], aexp2[:],
                                            idm[0:96, 0:96])
                        atT = tp.tile([96, 96], BF16, name=f"atT{h}")
                        nc.scalar.copy(atT[:], at_ps[:])
                        atTs[h] = atT

                # ============ attn@v + regroup + proj ============
                with tc.tile_pool(name="attn", bufs=2) as ap, \
                     tc.tile_pool(name="psum_attn", bufs=2,
                                  space="PSUM") as pap:
                    for ck in range(NCK):
                        z = ap.tile([48, 4, 4, W], BF16, tag="z")
                        for h in range(HEADS):
                            av_ps = pap.tile([96, W], F32, tag="av_ps",
                                             bufs=2)
                            rhs = vcbs[h][:, 4 * ck:4 * ck + 4, :].rearrange(
                                "p a b -> p (a b)")
                            nc.tensor.matmul(av_ps[:], atTs[h][:], rhs,
                                             start=True, stop=True)
                            och = ap.tile([96, W], BF16, tag="och", bufs=4)
                            if h % 2 == 0:
                                nc.scalar.copy(och[:], av_ps[:])
                            else:
                                nc.vector.tensor_copy(och[:], av_ps[:])
                            nc.scalar.dma_start(
                                out=z[6 * h:6 * h + 6], in_=och[:])
                        for nh in range(4):
                            ystg = ap.tile([48, 4, W], F32, tag="ystg",
                                           bufs=4)
                            for nw in range(4):
                                pj = pap.tile([48, W], F32, tag="pj", bufs=2)
                                nc.tensor.matmul(pj[:], wp[:],
                                                 z[0:48, nh, nw, :],
                                                 start=True, stop=True)
                                nc.scalar.copy(ystg[:, nw, :], pj[:])
                            nc.sync.dma_start(
                                out=y_out[:, nh, :, 4 * ck:4 * ck + 4, :],
                                in_=ystg[:])

    return _finish(nc)


def _finish(nc):
    nc.compile()
    nc.generate_event_semaphores()
    return nc


# ====================== host-side helpers ======================

def host_prepare(x, qkv_w, dw_w, proj_w, cfg: Cfg):
    """Full inputs -> list of per-core input dicts."""
    bf = ml_dtypes.bfloat16
    b, cin, H, Wf = x.shape
    assert cin == CIN and Wf == cfg.W
    n = cfg.n_cores
    per_b = n // b
    assert cfg.HLOC * per_b == H

    w1 = qkv_w[:, :, 0, 0].astype(np.float32)          # [144, 48]
    dw = dw_w[:, 0, :, :].astype(np.float32)           # [144, 3, 3]
    # w3T[ic, dy, dx, oc] = w1[oc, ic] * dw[oc, dy, dx]
    w3T = np.einsum('oi,oyx->iyxo', w1, dw)            # [48, 3, 3, 144]

    w3a = np.zeros((128, 3, 96), np.float32)
    w3av = np.zeros((128, 3, 48), np.float32)
    for dy in range(3):
        w3a[0:48, dy] = w3T[:, dy, 0, 0:96]
        w3a[48:96, dy] = w3T[:, dy, 1, 0:96]
        w3a[96:128, dy] = w3T[0:32, dy, 2, 0:96]
        w3av[0:48, dy] = w3T[:, dy, 0, 96:144]
        w3av[48:96, dy] = w3T[:, dy, 1, 96:144]
        w3av[96:128, dy] = w3T[0:32, dy, 2, 96:144]
    w3b = np.zeros((48, 96), np.float32)
    w3bv = np.zeros((48, 48), np.float32)
    for dy in range(3):
        w3b[16 * dy:16 * dy + 16] = w3T[32:48, dy, 2, 0:96]
        w3bv[16 * dy:16 * dy + 16] = w3T[32:48, dy, 2, 96:144]

    wp = np.ascontiguousarray(proj_w[:, :, 0, 0].astype(np.float32).T)
    idm = np.eye(128, dtype=np.float32)

    xp = np.zeros((b, cin, H + 2, cfg.W + 2), dtype=np.float32)
    xp[:, :, 1:H + 1, 1:cfg.W + 1] = x
    xp = xp.astype(bf)

    const = {"w3a": w3a.astype(bf), "w3av": w3av.astype(bf),
             "w3b": w3b.astype(bf), "w3bv": w3bv.astype(bf),
             "wp": wp.astype(bf), "idm": idm.astype(bf)}
    in_maps = []
    for i in range(n):
        bi, r = i // per_b, i % per_b
        xs = np.ascontiguousarray(
            xp[bi, :, cfg.HLOC * r:cfg.HLOC * r + cfg.HLOC + 2, :])
        m = {"x": xs}
        m.update(const)
        in_maps.append(m)
    return in_maps


def host_assemble(results, cfg: Cfg, b, H):
    n = cfg.n_cores
    per_b = n // b
    y = np.empty((b, CIN, H, cfg.W), dtype=np.float32)
    for i in range(n):
        bi, r = i // per_b, i % per_b
        yb = results[i]["y"]                     # [48, 4, 4, 32, 128]
        yc = yb.transpose(0, 3, 1, 4, 2).reshape(CIN, cfg.HLOC, cfg.W)
        y[bi, :, cfg.HLOC * r:cfg.HLOC * (r + 1), :] = yc
    return y


_NC_CACHE = {}


def kernel(x, qkv_w, dw_w, proj_w, temperature):
    x = np.asarray(x, dtype=np.float32)
    qkv_w = np.asarray(qkv_w, dtype=np.float32)
    dw_w = np.asarray(dw_w, dtype=np.float32)
    proj_w = np.asarray(proj_w, dtype=np.float32)
    temperature = np.asarray(temperature, dtype=np.float32)

    from concourse.bass_utils import run_bass_kernel_spmd

    cfg = Cfg(W=512, HLOC=128, n_cores=8)
    temps = tuple(float(t) for t in temperature[:, 0, 0])
    if temps not in _NC_CACHE:
        _NC_CACHE[temps] = build_nc(list(temps), cfg.n_cores)
    nc = _NC_CACHE[temps]
    in_maps = host_prepare(x, qkv_w, dw_w, proj_w, cfg)
    res = run_bass_kernel_spmd(nc, in_maps, list(range(cfg.n_cores)))
    return host_assemble(res.results, cfg, x.shape[0], x.shape[2])
